# revision 29
# baseline (speedup 1.0000x reference)
"""3-layer GAT on 8 Trainium2 NeuronCores.

Strategy (dst-sharded, edge segments on the DVE free dim):
- Nodes are sharded by dst across 8 cores (6250/core, padded to 6272 = 49*128).
  Within a core, dsts are sorted by in-degree and chunked into 49 groups of 128
  (group t, partition p). Each group gets a uniform per-dst edge budget g_t
  (max degree in group, rounded even), shared across all cores so the SPMD
  program is identical.
- Per layer: each core computes ft/el/er for its own rows (dense matmul on PE),
  writes an fp16 feature table shard [6272, RW], AllGathers the full table
  [50176, RW], then bulk-gathers per-edge rows ft[src] with dma_gather.
- dma_gather indices are int16, so two overlapping 32768-row windows cover the
  table: L=[0,32768) and H=[ROWS-32768,ROWS). Rows in the overlap are assigned
  per-dst to balance low/high slot counts, and nodes are ordered by (low,high)
  so per-group budgets are tight (~1.17x padding vs 1.80x for a hard split).
  Pad edge slots point at a pad row whose el is -1e30 (=> exp contribution 0).
- Edge softmax: e = lrelu(el[src]+er[dst]); ee = exp(e) in bf16 (bf16 has f32
  range, so no max-shift stabilization is needed — any per-dst constant shift
  cancels between numerator and denominator anyway). Denominators via
  segmented reduce, unnormalized aggregation ms = ft*ee reduced per dst
  (contiguous [g, hd] halving tree), then scaled by 1/den at the end.
"""
import sys, os, types
sys.path.insert(0, "/opt/trn_rl_repo")
import numpy as np
import ml_dtypes


def _install_ntff_hook():
    """Provide antenv.axon_hooks so run_bass_kernel_spmd(trace=True) can
    profile via NTFF on this axon-tunneled setup."""
    try:
        import antenv
        if "antenv.axon_hooks" in sys.modules:
            return
        mod = types.ModuleType("antenv.axon_hooks")
        _h = [None]
        mod.set_axon_ntff_profile_hook = lambda h: _h.__setitem__(0, h)
        mod.get_axon_ntff_profile_hook = lambda: _h[0]
        sys.modules["antenv.axon_hooks"] = mod
        antenv.axon_hooks = mod
        from trn_agent_boot.trn_boot import _ntff_profile_via_ctypes
        mod.set_axon_ntff_profile_hook(
            _ntff_profile_via_ctypes("/opt/axon/libaxon_pjrt.so"))
    except Exception:
        pass


_install_ntff_hook()

from concourse import bass, bacc, tile, mybir
from concourse.bass import AP
from concourse.bass_utils import run_bass_kernel_spmd
from concourse.masks import make_identity

F32 = mybir.dt.float32
F16 = mybir.dt.float16
BF16 = mybir.dt.bfloat16
I16 = mybir.dt.int16
I32 = mybir.dt.int32

P = 128
NCORES = 8

LAST_EXEC_NS = None    # set when GAT_TRACE=1
LAST_RESULT = None


# ---------------------------------------------------------------------------
# configuration
# ---------------------------------------------------------------------------
class Cfg:
    def __init__(self, N, E, IN, D, C, heads, split=32768):
        self.N, self.E, self.IN, self.D, self.C = N, E, IN, D, C
        self.split = split
        self.heads = heads
        self.S = N // NCORES                 # real nodes per core
        self.T = (self.S + P - 1) // P       # groups per core
        self.SP = self.T * P                 # padded nodes per core
        self.ROWS = NCORES * self.SP         # table rows
        # per-layer head count / per-head feat / table fp16 slots
        h0, h1, h2 = heads
        self.l_heads = [h0, h1, h2]
        self.l_fd = [D, D, C]
        self.l_hd = [h0 * D, h1 * D, h2 * C]
        self.l_in = [IN, h0 * D, h1 * D]
        # table row: hd fp16 + h f32 (2 slots each), padded to 128-slot mult
        self.l_rw = []
        for l in range(3):
            raw = self.l_hd[l] + 2 * self.l_heads[l]
            self.l_rw.append(((raw + 127) // 128) * 128)
        # dense matmul output width: hd + h(el) + h(er) [+ res for l2]
        self.l_nw = [self.l_hd[l] + 2 * self.l_heads[l] for l in range(3)]
        self.l_nw[2] += self.l_hd[2]


FULL = Cfg(N=50000, E=800000, IN=256, D=64, C=32, heads=(4, 4, 6))
SLOPE = 0.2
C_MAX = int(os.environ.get("GAT_CMAX", "21"))   # gather chunk columns
G_CAP = 16            # max span width per dst group


# ---------------------------------------------------------------------------
# host-side graph preprocessing (indices only)
# ---------------------------------------------------------------------------
def preprocess(cfg, src, dst):
    """Slot scheduling with overlapping int16 windows.

    Structure L covers table rows [0, 32768); structure H covers rows
    [ROWS-32768, ROWS). Rows in the overlap [ROWS-32768, 32768) are "flex"
    and are assigned per-dst to whichever side balances its low/high slot
    counts. Nodes are then sorted per core by (low, high) desc so each
    128-group's budget (max over lanes and cores) is tight.
    """
    N, S, SP, T = cfg.N, cfg.S, cfg.SP, cfg.T
    ROWS = cfg.ROWS
    WIN = 32768
    AH_BASE = ROWS - WIN                     # 17408 for full cfg
    assert AH_BASE >= 0 and AH_BASE < WIN, "windows must cover the table"

    e_core = dst // S

    # per-core balanced (low, high) targets and node ordering
    perms = []
    rank_of = np.zeros(N, np.int64)
    low_of = np.zeros(N, np.int64)           # per dst: #slots on L side
    for c in range(NCORES):
        m = e_core == c
        nid = dst[m] - c * S
        cd = np.bincount(nid, minlength=S)
        # class of each edge by src row will be known only after ALL cores'
        # perms exist (src rows depend on every core's ordering).  Break the
        # circularity: the window boundaries in *rank space* are fixed
        # (row = core*SP + rank), so classify by the src's core/rank which we
        # compute in a first pass below.  To keep it simple we do two passes:
        # pass 1 orders nodes by degree only (approx), pass 2 reorders by
        # (low, high).  Instead, classify by src core only is not enough —
        # so we defer: store degree for now.
        perms.append(np.arange(S) + c * S)
        rank_of[c * S:(c + 1) * S] = np.arange(S)

    # pass 1: tentative row numbers with identity perm (class of an edge
    # depends only on its src row vs the two window boundaries; rows of a
    # core's nodes stay within [c*SP, c*SP+S) under any within-core perm,
    # so an edge's class can only change if its src row straddles a window
    # boundary *within* the same core's range.  Boundaries 17408 and 32768
    # fall inside core 2 (rows 12544..18816) and core 5 (rows 31360..37632).
    # For those cores the class depends on the final rank, so iterate twice.
    def classify_and_balance():
        node_row = (np.arange(N) // S) * SP + rank_of
        e_row = node_row[src]
        mustL = e_row < AH_BASE
        mustH = e_row >= WIN
        low_all = np.zeros(N, np.int64)
        high_all = np.zeros(N, np.int64)
        for c in range(NCORES):
            m = e_core == c
            nid = dst[m] - c * S
            cd = np.bincount(nid, minlength=S)
            cl = np.bincount(nid[mustL[m]], minlength=S)
            ch = np.bincount(nid[mustH[m]], minlength=S)
            low = np.clip((cd + 1) // 2, cl, cd - ch)
            low_all[c * S:(c + 1) * S] = low
            high_all[c * S:(c + 1) * S] = cd - low
        return low_all, high_all

    for _ in range(3):
        low_all, high_all = classify_and_balance()
        perms = []
        for c in range(NCORES):
            lo = low_all[c * S:(c + 1) * S]
            hi = high_all[c * S:(c + 1) * S]
            # primary: max(lo,hi) desc; secondary: lo-hi asc — clusters
            # same-magnitude nodes and separates L-heavy from H-heavy
            order = np.lexsort((lo - hi, -np.maximum(lo, hi)))
            perms.append(order + c * S)
            rank_of[c * S + order] = np.arange(S)

    low_all, high_all = classify_and_balance()
    node_row = (np.arange(N) // S) * SP + rank_of
    e_row = node_row[src]
    e_rank = rank_of[dst]

    # shared budgets (max over lanes and cores), no even rounding
    gL = np.zeros(T, np.int64)
    gH = np.zeros(T, np.int64)
    for c in range(NCORES):
        lo = low_all[perms[c]]
        hi = high_all[perms[c]]
        lop = np.zeros(SP, np.int64); lop[:S] = lo
        hip = np.zeros(SP, np.int64); hip[:S] = hi
        gL = np.maximum(gL, lop.reshape(T, P).max(axis=1))
        gH = np.maximum(gH, hip.reshape(T, P).max(axis=1))
    AL = np.concatenate([[0], np.cumsum(gL)])
    AH = np.concatenate([[0], np.cumsum(gH)])
    GL, GH = int(AL[-1]), int(AH[-1])

    PADL = cfg.SP - 1                       # core0 last dummy row (< WIN)
    PADH = WIN - 1                          # row ROWS-1 (core7 dummy) - AH_BASE
    assert cfg.S < cfg.SP, "need dummy rows for pad slots"
    assert PADL < WIN and ROWS - 1 - AH_BASE == PADH

    # slot arrays [NCORES][128, GL/GH] int32 (values < 32768)
    slotsL = np.full((NCORES, P, GL), PADL, np.int32)
    slotsH = np.full((NCORES, P, GH), PADH, np.int32)
    # fill: order edges by (core, rank, class) where class: mustL=0, flex=1,
    # mustH=2.  Within a dst the first low_of slots go to L, rest to H.
    e_class = np.ones(len(src), np.int8)
    e_class[e_row < AH_BASE] = 0
    e_class[e_row >= WIN] = 2
    for c in range(NCORES):
        m = e_core == c
        r = e_rank[m]
        rows = e_row[m]
        cls = e_class[m]
        o = np.lexsort((cls, r))
        r, rows = r[o], rows[o]
        starts = np.searchsorted(r, np.arange(SP))
        j = np.arange(len(r)) - starts[r]
        lo_r = low_all[dst[m][o]]
        is_l = j < lo_r
        rl, jl, rowl = r[is_l], j[is_l], rows[is_l]
        rh, jh, rowh = r[~is_l], j[~is_l] - lo_r[~is_l], rows[~is_l] - AH_BASE
        slotsL[c][rl % P, AL[rl // P] + jl] = rowl
        slotsH[c][rh % P, AH[rh // P] + jh] = rowh
    assert slotsL.max() < WIN and slotsL.min() >= 0
    assert slotsH.max() < WIN and slotsH.min() >= 0

    # spans: split each group's column range into pieces <= G_CAP
    def build_spans(g_sched, A):
        spans = []   # (t, g_span, col0_in_struct)
        for t in range(T):
            g = int(g_sched[t])
            base = int(A[t])
            off = 0
            while g > 0:
                npc = -(-g // G_CAP)
                s = min(-(-g // npc), g)
                spans.append((t, s, base + off))
                off += s
                g -= s
        return spans

    # chunks hold only equal-g spans so the edge phase can express the
    # mult / halving tree / den reduce as single 3D-AP instructions with
    # the span index as a dimension: (struct, g, [(t, col0_in_struct)...])
    def build_chunks(spans, struct):
        by_g = {}
        for (t, g, c0) in spans:
            by_g.setdefault(g, []).append((t, c0))
        chunks = []
        for g in sorted(by_g, reverse=True):
            lst = by_g[g]
            nb_max = max(1, C_MAX // g)
            for i in range(0, len(lst), nb_max):
                chunks.append((struct, g, lst[i:i + nb_max]))
        return chunks

    chunks = (build_chunks(build_spans(gL, AL), 0)
              + build_chunks(build_spans(gH, AH), 1))

    # wrapped int16 index tiles [NCORES][128, (GL+GH)*8]
    def wrap(slots_c, chunk_list, Gtot):
        outw = np.zeros((P, Gtot * 8), np.int16)
        pos = 0
        for (struct, g, spans_l) in chunk_list:
            s = slots_c[1] if struct else slots_c[0]
            cols = np.concatenate([np.arange(c0, c0 + g)
                                   for (_t, c0) in spans_l])
            blk = s[:, cols]                       # [128, ncols]
            ncols = len(cols)
            flat = blk.T.reshape(-1)               # n = col*128 + p
            w = flat.reshape(-1, 16).T             # [16, n/16]
            w = np.tile(w, (8, 1)).astype(np.int16)
            outw[:, pos:pos + ncols * 8] = w
            pos += ncols * 8
        assert pos == Gtot * 8
        return outw

    Gtot = GL + GH
    idx_tiles = [wrap((slotsL[c], slotsH[c]), chunks, Gtot)
                 for c in range(NCORES)]

    return dict(perms=perms, gL=gL, gH=gH, AL=AL, AH=AH, GL=GL, GH=GH,
                chunks=chunks, idx_tiles=idx_tiles, PADL=PADL, PADH=PADH,
                WIN=WIN, AH_BASE=AH_BASE)


# ---------------------------------------------------------------------------
# bass program
# ---------------------------------------------------------------------------
def build_program(cfg, pp):
    T, SP, ROWS = cfg.T, cfg.SP, cfg.ROWS
    WIN, AH_BASE = pp["WIN"], pp["AH_BASE"]
    chunks = pp["chunks"]
    Gtot = pp["GL"] + pp["GH"]
    assert all(cfg.l_in[l] == cfg.IN for l in range(3))

    NSWQ = int(os.environ.get("GAT_SWQ", "2"))
    nc = bacc.Bacc("TRN2", target_bir_lowering=False, debug=False,
                   num_devices=NCORES, num_swdge_queues=NSWQ)

    # --- external tensors ---
    xT_d = nc.dram_tensor("xT", [cfg.IN, SP], F16, kind="ExternalInput")
    idx_d = nc.dram_tensor("idx", [P, Gtot * 8], I16, kind="ExternalInput")
    w_d = [nc.dram_tensor(f"W{l}", [cfg.l_in[l], cfg.l_nw[l]], F16,
                          kind="ExternalInput") for l in range(3)]
    b_d = [nc.dram_tensor(f"b{l}", [1, cfg.l_hd[l]], F32,
                          kind="ExternalInput") for l in range(3)]
    pad_d = [nc.dram_tensor(f"pad{l}", [1, cfg.l_rw[l]], BF16,
                            kind="ExternalInput") for l in range(3)]
    out_d = nc.dram_tensor("out", [SP, cfg.C], F32, kind="ExternalOutput")

    # --- internal DRAM ---
    shard_d = [nc.dram_tensor(f"shard{l}", [SP, cfg.l_rw[l]], BF16)
               for l in range(3)]
    table_d = [nc.dram_tensor(f"table{l}", [ROWS, cfg.l_rw[l]], BF16,
                              addr_space="Shared") for l in range(3)]
    res_d = nc.dram_tensor("resbuf", [SP, cfg.l_hd[2]], F32)

    IN_CH = cfg.IN // P      # contraction chunks (2 for full)
    assert cfg.IN % P == 0

    GBUFS = int(os.environ.get("GAT_GBUFS", "3"))
    with tile.TileContext(nc) as tc:
        sb = tc.alloc_tile_pool(name="sb", bufs=1)
        lay = tc.alloc_tile_pool(name="lay", bufs=2)
        erp = tc.alloc_tile_pool(name="erp", bufs=1)
        trans = tc.alloc_tile_pool(name="trans", bufs=2)
        gpool = tc.alloc_tile_pool(name="gath", bufs=GBUFS)
        mspool = tc.alloc_tile_pool(name="ms", bufs=1)
        xpool = tc.alloc_tile_pool(name="eex", bufs=2)
        epool = tc.alloc_tile_pool(name="e2", bufs=GBUFS + 1)
        hpool = tc.alloc_tile_pool(name="h16", bufs=2)
        rawpool = tc.alloc_tile_pool(name="hraw", bufs=1)
        opool = tc.alloc_tile_pool(name="outs", bufs=2)
        psum = tc.alloc_tile_pool(name="ps", bufs=2, space="PSUM")
        pst = tc.alloc_tile_pool(name="pst", bufs=2, space="PSUM")

        ident = sb.tile([P, P], F16)
        make_identity(nc, ident[:])

        PHASE = os.environ.get("GAT_PHASE", "full")
        NLAYERS = int(os.environ.get("GAT_LAYERS", "3"))
        h_prev = None
        for l in range(NLAYERS):
            nh, fd, hd = cfg.l_heads[l], cfg.l_fd[l], cfg.l_hd[l]
            nw, rw = cfg.l_nw[l], cfg.l_rw[l]
            rwf = rw // 2            # f32 view row length
            elo = hd // 2            # el offset in f32 view

            w_sb = lay.tile([P, IN_CH * nw], F16, tag="w")
            for k in range(IN_CH):
                nc.sync.dma_start(out=w_sb[:, k * nw:(k + 1) * nw],
                                  in_=w_d[l][k * P:(k + 1) * P, :])
            bias_sb = lay.tile([P, hd], F32, tag="bias")
            bd = b_d[l][:]
            nc.sync.dma_start(
                out=bias_sb[:],
                in_=AP(bd.tensor, 0, [[0, P], [1, hd]]))

            er_sb = lay.tile([P, T * nh], F32, tag="er")

            # ---------------- dense phase ----------------
            for t in range(T):
                lhs = []
                for k in range(IN_CH):
                    hT = trans.tile([P, P], F16, tag="hT")
                    if l == 0:
                        nc.sync.dma_start(
                            out=hT[:],
                            in_=xT_d[k * P:(k + 1) * P, t * P:(t + 1) * P])
                    else:
                        tp = pst.tile([P, P], F16, tag="tp")
                        nc.tensor.transpose(
                            out=tp[:],
                            in_=h_prev[:, t * cfg.l_in[l] + k * P:
                                       t * cfg.l_in[l] + (k + 1) * P],
                            identity=ident[:])
                        nc.scalar.copy(out=hT[:], in_=tp[:])
                    lhs.append(hT)
                ps_ft = psum.tile([P, nw], F32, tag="ft")
                for k in range(IN_CH):
                    nc.tensor.matmul(out=ps_ft[:], lhsT=lhs[k][:],
                                     rhs=w_sb[:, k * nw:(k + 1) * nw],
                                     start=(k == 0), stop=(k == IN_CH - 1))
                ftx = trans.tile([P, rw], BF16, tag="ftx")
                if rw > hd + 2 * nh:
                    nc.vector.memset(ftx[:, hd + 2 * nh:], 0.0)
                nc.vector.tensor_copy(out=ftx[:, :hd], in_=ps_ft[:, :hd])
                # el (f32) into bf16 slot pairs [hd : hd+2*nh)
                nc.vector.tensor_copy(
                    out=ftx[:, hd:hd + 2 * nh].bitcast(F32),
                    in_=ps_ft[:, hd:hd + nh])
                nc.vector.tensor_copy(out=er_sb[:, t * nh:(t + 1) * nh],
                                      in_=ps_ft[:, hd + nh:hd + 2 * nh])
                if l == 2:
                    rst = trans.tile([P, hd], F32, tag="rst")
                    nc.scalar.copy(out=rst[:],
                                   in_=ps_ft[:, hd + 2 * nh:nw])
                    nc.sync.dma_start(out=res_d[t * P:(t + 1) * P, :],
                                      in_=rst[:])
                nc.sync.dma_start(out=shard_d[l][t * P:(t + 1) * P, :],
                                  in_=ftx[:])

            if PHASE == "dense":
                continue
            # ---------------- allgather + pad rows ----------------
            nc.gpsimd.collective_compute(
                "AllGather", mybir.AluOpType.bypass,
                replica_groups=[list(range(NCORES))],
                ins=[shard_d[l][:]], outs=[table_d[l][:]])
            padrow = trans.tile([1, rw], BF16, tag="padr")
            nc.sync.dma_start(out=padrow[:], in_=pad_d[l][:])
            nc.sync.dma_start(out=table_d[l][pp["PADL"]:pp["PADL"] + 1, :],
                              in_=padrow[:])
            nc.sync.dma_start(out=table_d[l][ROWS - 1:ROWS, :],
                              in_=padrow[:])

            if PHASE == "ag":
                continue
            # ---------------- edge phase ----------------
            # er_exp: er broadcast per slot column, in chunk-column order
            er_exp = erp.tile([P, Gtot * nh], F32, tag="erx")
            erv = er_exp[:]
            pos = 0
            for (struct, g, spans_l) in chunks:
                for (t, _c0) in spans_l:
                    o = AP(erv.tensor, erv.offset + pos * nh,
                           [erv.ap[0], [nh, g], [1, nh]])
                    i = AP(er_sb[:].tensor, er_sb[:].offset + t * nh,
                           [er_sb[:].ap[0], [0, g], [1, nh]])
                    nc.vector.tensor_copy(out=o, in_=i)
                    pos += g

            h_raw = rawpool.tile([P, T * hd], BF16, tag="hraw")
            den_acc = lay.tile([P, T * nh], F32, tag="den")
            nc.vector.memset(h_raw[:], 0.0)
            nc.vector.memset(den_acc[:], 0.0)
            idx_pos = 0
            cbase = 0
            for ci, (struct, g, spans_l) in enumerate(chunks):
                nb = len(spans_l)
                ncols = g * nb
                n_idx = ncols * P
                idx_sb = epool.tile([P, ncols * 8], I16, tag="idx")
                nc.sync.dma_start(out=idx_sb[:],
                                  in_=idx_d[:, idx_pos:idx_pos + ncols * 8])
                gt = gpool.tile([P, ncols * rw], BF16, tag="g")
                gta = gt[:]
                out3 = AP(gta.tensor, gta.offset,
                          [gta.ap[0], [rw, ncols], [1, rw]])
                if struct == 0:
                    src_ap = table_d[l][0:WIN, :]
                else:
                    src_ap = table_d[l][AH_BASE:ROWS, :]
                nc.gpsimd.dma_gather(
                    out_ap=out3, in_ap=src_ap, idxs_ap=idx_sb[:],
                    num_idxs=n_idx, num_idxs_reg=n_idx, elem_size=rw,
                    single_packet=False, queue_num=ci % NSWQ)
                idx_pos += ncols * 8
                gf = gta.bitcast(F32)
                # whole-chunk e = lrelu(el + er); exp on Scalar, written
                # pre-broadcast to hd width so the ms mult is all-stride-1
                # bf16 (DVE 4x mode)
                e2 = epool.tile([P, ncols * nh], F32, tag="e2")
                e2t = epool.tile([P, ncols * nh], F32, tag="e2t")
                eex = xpool.tile([P, C_MAX * hd], BF16, tag="eex")
                el_ap = AP(gf.tensor, gf.offset + elo,
                           [gf.ap[0], [rwf, ncols], [1, nh]])
                nc.vector.tensor_tensor(
                    out=e2[:], in0=el_ap,
                    in1=erv[:, cbase * nh:(cbase + ncols) * nh],
                    op=mybir.AluOpType.add)
                nc.vector.tensor_scalar_mul(e2t[:], e2[:], SLOPE)
                nc.vector.tensor_tensor(out=e2[:], in0=e2[:], in1=e2t[:],
                                        op=mybir.AluOpType.max)
                eexv = eex[:]
                eex_out = AP(eexv.tensor, eexv.offset,
                             [eexv.ap[0], [hd, ncols], [fd, nh], [1, fd]])
                e2_in = AP(e2[:].tensor, e2[:].offset,
                           [e2[:].ap[0], [nh, ncols], [1, nh], [0, fd]])
                nc.scalar.activation(eex_out, e2_in,
                                     mybir.ActivationFunctionType.Exp)
                cbase += ncols
                # den partials: one strided reduce for all nb spans
                eea = AP(eexv.tensor, eexv.offset,
                         [eexv.ap[0], [g * hd, nb], [fd, nh], [hd, g]])
                dtmp = epool.tile([P, nb * nh], F32, tag="dtmp")
                nc.vector.tensor_reduce(out=dtmp[:], in_=eea,
                                        axis=mybir.AxisListType.X,
                                        op=mybir.AluOpType.add)
                for i, (t, _c0) in enumerate(spans_l):
                    nc.vector.tensor_tensor(
                        out=den_acc[:, t * nh:(t + 1) * nh],
                        in0=den_acc[:, t * nh:(t + 1) * nh],
                        in1=dtmp[:, i * nh:(i + 1) * nh],
                        op=mybir.AluOpType.add)
                # ms = ft * ee for the whole chunk, then one halving tree
                # over all spans (layout [nb, g, hd], hd contiguous)
                ms = mspool.tile([P, C_MAX * hd], BF16, tag="ms")
                msv = ms[:]
                msa = AP(msv.tensor, msv.offset,
                         [msv.ap[0], [hd, ncols], [fd, nh], [1, fd]])
                ft2_ap = AP(gta.tensor, gta.offset,
                            [gta.ap[0], [rw, ncols], [fd, nh], [1, fd]])
                ee2_ap = AP(eexv.tensor, eexv.offset,
                            [eexv.ap[0], [hd, ncols], [fd, nh], [1, fd]])
                nc.vector.tensor_tensor(out=msa, in0=ft2_ap, in1=ee2_ap,
                                        op=mybir.AluOpType.mult)
                with nc.allow_low_precision("bf16 h_raw"):
                    gg = g
                    while gg > 1:
                        g2 = gg // 2
                        dst = AP(msv.tensor, msv.offset,
                                 [msv.ap[0], [g * hd, nb], [hd, g2],
                                  [1, hd]])
                        srp = AP(msv.tensor, msv.offset + g2 * hd,
                                 [msv.ap[0], [g * hd, nb], [hd, g2],
                                  [1, hd]])
                        nc.vector.tensor_tensor(out=dst, in0=dst, in1=srp,
                                                op=mybir.AluOpType.add)
                        if gg % 2:
                            d1 = AP(msv.tensor, msv.offset,
                                    [msv.ap[0], [g * hd, nb], [1, hd]])
                            s1 = AP(msv.tensor, msv.offset + (gg - 1) * hd,
                                    [msv.ap[0], [g * hd, nb], [1, hd]])
                            nc.vector.tensor_tensor(out=d1, in0=d1, in1=s1,
                                                    op=mybir.AluOpType.add)
                        gg = g2
                    for i, (t, _c0) in enumerate(spans_l):
                        m0 = AP(msv.tensor, msv.offset + i * g * hd,
                                [msv.ap[0], [1, hd]])
                        nc.vector.tensor_tensor(
                            out=h_raw[:, t * hd:(t + 1) * hd],
                            in0=h_raw[:, t * hd:(t + 1) * hd],
                            in1=m0, op=mybir.AluOpType.add)

            if PHASE == "edge":
                continue
            # ---------------- normalize + output ----------------
            nc.vector.tensor_scalar_max(den_acc[:], den_acc[:], 1e-30)
            rec = lay.tile([P, T * nh], F32, tag="rec")
            nc.vector.reciprocal(rec[:], den_acc[:])

            if l < 2:
                h_next = hpool.tile([P, T * hd], F16, tag="h16")
            else:
                h_next = None
            NT = int(os.environ.get("GAT_NT", "2"))
            for t0 in range(0, T, NT):
                nt = min(NT, T - t0)
                o32 = opool.tile([P, NT * hd], F32, tag="o32")
                ha = h_raw[:, t0 * hd:(t0 + nt) * hd]
                o32a = AP(o32[:].tensor, o32[:].offset,
                          [o32[:].ap[0], [hd, nt], [fd, nh], [1, fd]])
                h_ap = AP(ha.tensor, ha.offset,
                          [ha.ap[0], [hd, nt], [fd, nh], [1, fd]])
                rec_ap = AP(rec[:].tensor, rec[:].offset + t0 * nh,
                            [rec[:].ap[0], [nh, nt], [1, nh], [0, fd]])
                nc.vector.tensor_tensor(out=o32a, in0=h_ap, in1=rec_ap,
                                        op=mybir.AluOpType.mult)
                bias_ap = AP(bias_sb[:].tensor, bias_sb[:].offset,
                             [bias_sb[:].ap[0], [0, nt], [1, hd]])
                ow = o32[:, :nt * hd]
                o2 = AP(ow.tensor, ow.offset, [ow.ap[0], [hd, nt], [1, hd]])
                nc.vector.tensor_tensor(out=o2, in0=o2, in1=bias_ap,
                                        op=mybir.AluOpType.add)
                if l == 1:
                    nc.vector.tensor_tensor(
                        out=ow, in0=ow,
                        in1=h_prev[:, t0 * hd:(t0 + nt) * hd],
                        op=mybir.AluOpType.add)
                if l == 2:
                    rl = opool.tile([P, NT * hd], F32, tag="rl")
                    rla = rl[:, :nt * hd]
                    nc.sync.dma_start(
                        out=rla,
                        in_=AP(res_d[:].tensor, t0 * P * hd,
                               [[hd, P], [P * hd, nt], [1, hd]]))
                    nc.vector.tensor_tensor(out=ow, in0=ow, in1=rla,
                                            op=mybir.AluOpType.add)
                    # mean over heads -> [P, nt*C]
                    om = opool.tile([P, NT * cfg.C], F32, tag="om")
                    oma = om[:, :nt * cfg.C]
                    in_ap = AP(ow.tensor, ow.offset,
                               [ow.ap[0], [hd, nt], [1, fd], [fd, nh]])
                    out_ap = AP(oma.tensor, oma.offset,
                                [oma.ap[0], [cfg.C, nt], [1, fd]])
                    nc.vector.tensor_reduce(out=out_ap, in_=in_ap,
                                            axis=mybir.AxisListType.X,
                                            op=mybir.AluOpType.add)
                    nc.vector.tensor_scalar_mul(oma, oma, 1.0 / nh)
                    nc.sync.dma_start(
                        out=AP(out_d[:].tensor, t0 * P * cfg.C,
                               [[cfg.C, P], [P * cfg.C, nt], [1, cfg.C]]),
                        in_=oma)
                else:
                    # elu: q=relu(-x); x=relu(x); q=exp(-q); x=x+q; h=x-1
                    q = opool.tile([P, NT * hd], F32, tag="q")
                    qa = q[:, :nt * hd]
                    nc.scalar.activation(qa, ow,
                                         mybir.ActivationFunctionType.Relu,
                                         scale=-1.0)
                    nc.scalar.activation(ow, ow,
                                         mybir.ActivationFunctionType.Relu)
                    nc.scalar.activation(qa, qa,
                                         mybir.ActivationFunctionType.Exp,
                                         scale=-1.0)
                    nc.vector.tensor_tensor(out=ow, in0=ow, in1=qa,
                                            op=mybir.AluOpType.add)
                    nc.vector.tensor_scalar_add(
                        h_next[:, t0 * hd:(t0 + nt) * hd], ow, -1.0)
            h_prev = h_next

        if PHASE != "full" or NLAYERS < 3:
            z = trans.tile([P, cfg.C], F32, tag="zout")
            nc.vector.memset(z[:], 0.0)
            for t in range(T):
                nc.sync.dma_start(
                    out=AP(out_d[:].tensor, t * P * cfg.C,
                           [[cfg.C, P], [1, cfg.C]]),
                    in_=z[:])
        for pool in (pst, psum, opool, rawpool, hpool, epool, xpool, mspool, gpool,
                     trans, erp, lay, sb):
            pool.release()

    nc.compile()
    return nc


# ---------------------------------------------------------------------------
# host wrapper
# ---------------------------------------------------------------------------
def _weights_full(cfg, W, al, ar, res=None):
    h, fd = al.shape
    Wl = np.einsum("ihd,hd->ih", W.reshape(W.shape[0], h, fd), al)
    Wr = np.einsum("ihd,hd->ih", W.reshape(W.shape[0], h, fd), ar)
    parts = [W, Wl, Wr]
    if res is not None:
        parts.append(res)
    return np.concatenate(parts, axis=1).astype(np.float16)


def _padrow(cfg, l):
    rw = cfg.l_rw[l]
    hd, nh = cfg.l_hd[l], cfg.l_heads[l]
    buf = np.zeros((1, rw), ml_dtypes.bfloat16)
    v = buf.view(np.uint8)
    v[0, 2 * hd:2 * hd + 4 * nh] = np.frombuffer(
        np.full(nh, -1e30, np.float32).tobytes(), np.uint8)
    return buf


_CACHE = {}


def run(cfg, inputs, trace=False):
    global LAST_EXEC_NS, LAST_RESULT
    x = inputs["x"]
    src = np.asarray(inputs["src"])
    dst = np.asarray(inputs["dst"])
    key = "prog"
    if key not in _CACHE:
        pp = preprocess(cfg, src, dst)
        nc = build_program(cfg, pp)
        _CACHE[key] = (pp, nc)
    pp, nc = _CACHE[key]

    h0, h1, h2 = cfg.heads
    wf = [_weights_full(cfg, np.asarray(inputs["W0"], np.float32),
                        np.asarray(inputs["al0"]), np.asarray(inputs["ar0"])),
          _weights_full(cfg, np.asarray(inputs["W1"], np.float32),
                        np.asarray(inputs["al1"]), np.asarray(inputs["ar1"])),
          _weights_full(cfg, np.asarray(inputs["W2"], np.float32),
                        np.asarray(inputs["al2"]), np.asarray(inputs["ar2"]),
                        np.asarray(inputs["resW2"], np.float32))]
    biases = [np.asarray(inputs["b0"], np.float32).reshape(1, -1),
              np.asarray(inputs["b1"], np.float32).reshape(1, -1),
              np.asarray(inputs["b2"], np.float32).reshape(1, -1)]
    pads = [_padrow(cfg, l) for l in range(3)]

    in_maps = []
    for c in range(NCORES):
        perm = pp["perms"][c]
        xp = np.zeros((cfg.SP, cfg.IN), np.float32)
        xp[:cfg.S] = np.asarray(x, np.float32)[perm]
        m = {"xT": np.ascontiguousarray(xp.T).astype(np.float16),
             "idx": pp["idx_tiles"][c]}
        for l in range(3):
            m[f"W{l}"] = wf[l]
            m[f"b{l}"] = biases[l]
            m[f"pad{l}"] = pads[l]
        in_maps.append(m)

    res = run_bass_kernel_spmd(nc, in_maps, list(range(NCORES)), trace=trace)
    LAST_RESULT = res
    LAST_EXEC_NS = res.exec_time_ns

    out = np.zeros((cfg.N, cfg.C), np.float32)
    for c in range(NCORES):
        out[pp["perms"][c]] = res.results[c]["out"][:cfg.S]
    return out


def kernel(**inputs):
    trace = os.environ.get("GAT_TRACE", "0") == "1"
    return run(FULL, inputs, trace=trace)

